# revision 17
# baseline (speedup 1.0000x reference)
"""CSwin vertical-stripe window attention (sparse_attention) on 8 TRN2 cores.

Sharding: data-parallel over batch B=8 (one image per NeuronCore). No
collectives. Per-core kernel computes windowed attention + output
projection for one [4096, 256] image; the tiny LePE depthwise 3x3 conv
(0.7% of FLOPs) is folded host-side into a per-window additive plane.

v6 design (exp split across Scalar AND Vector engines):
 - All input layouts prepared host-side: qT/kT/lepeT bf16, vn fp16 --
   one fused [128, 4096] DMA per window.
 - QK^T bf16, 4-head row-packed, split across TWO 2-bank PSUM tiles
   (heads 01 -> bigA, heads 23 -> bigB); fine-grained software pipeline
   emits, per jc-slot, this pair's QK+exp then the previous pair's
   PV/SM quarter and a proj piece, so no engine queue head ever blocks.
 - exp: Scalar-engine ACTIVATE for most tiles; for jc==1 tiles a
   single-instruction DVE Schraudolph (fp16-bit trick: round(x*a+b) as
   int16 IS the fp16 exp, ~3% max rel err, mean bias cancels in
   softmax) offloads ~20% of the exp work to the Vector engine.
 - PV + denominators col-packed fp16; reciprocal_approx_fast on DVE.
 - Window 7 (shifted stripes) is block-diagonal: masked quadrants are
   never computed (N=256 matmuls, strided Exp).
 - proj bias added by the DVE PSUM-evacuation op; output bf16
   window-major contiguous, host un-permutes.
"""
import numpy as np
import ml_dtypes

import concourse.bass as bass
import concourse.bacc as bacc
import concourse.mybir as mybir
import concourse.tile as tile

RESO, STRIPE, DIM, NH, HD = 64, 8, 256, 8, 32
B, L, WIN, NW = 8, RESO * RESO, RESO * STRIPE, RESO // STRIPE
P = 128
F32, BF16 = mybir.dt.float32, mybir.dt.bfloat16
F16, I16 = mybir.dt.float16, mybir.dt.int16

# fused per-window input blob offsets (16-bit elements, per partition)
O_QT, O_KT, O_VN, O_LP = 0, 1024, 2048, 3072
WCOLS = 4096

Exp = mybir.ActivationFunctionType.Exp
# Schraudolph fp16 exp: fp16_bits(e^x) ~ round(x * SCH_A + SCH_B)
SCH_A, SCH_B = 1477.3197218702985, 15315.5


def _dve_exp_half(w, g, jc, h0):
    """Which exp halves run on the Vector engine instead of Scalar."""
    if w == NW - 1 or jc != 1:
        return False
    return h0 == 2 or (w % 2 == 0)


def build_nc():
    nc = bacc.Bacc("TRN2", target_bir_lowering=False, debug=False)
    win = nc.declare_dram_parameter("win", [NW, P, WCOLS], BF16, isOutput=False)
    pw = nc.declare_dram_parameter("pw", [P, 2 * DIM], BF16, isOutput=False)
    pb = nc.declare_dram_parameter("pb", [P, DIM], F32, isOutput=False)
    out = nc.declare_dram_parameter("out", [L, DIM], BF16, isOutput=True)

    # output view: [w, p, (t4, c)] with window token t' = t4*128 + p
    ov = out[:].rearrange("(w p t) c -> w p (t c)", w=NW, p=P, t=4)

    with tile.TileContext(nc) as tc:
        with tc.tile_pool(name="const", bufs=1) as cp, \
             tc.tile_pool(name="sb", bufs=1) as sp, \
             tc.tile_pool(name="ps", bufs=1, space="PSUM") as pp:
            # ---- first window's qk plane goes out before anything else ----
            wts = {}
            wt0 = sp.tile([P, WCOLS], BF16, name="wt0", tag="wt", bufs=3)
            nc.sync.dma_start(wt0[:, :2048], win[:][0][:, :2048])
            nc.sync.dma_start(wt0[:, 2048:], win[:][0][:, 2048:])
            wts[0] = wt0

            # ---- constants ----
            ones32 = cp.tile([P, 32], F16, name="ones32")
            nc.vector.memset(ones32[:], 1.0)
            pw_sb = cp.tile([P, 2, DIM], BF16, name="pw_sb")
            nc.sync.dma_start(pw_sb[:], pw[:].rearrange("p (g c) -> p g c", g=2))
            pb_sb = cp.tile([P, DIM], F32, name="pb_sb")
            nc.sync.dma_start(pb_sb[:], pb[:])

            def views(wt):
                return (
                    wt[:, O_QT:O_QT + 1024].rearrange("p (g q) -> p g q", g=2),
                    wt[:, O_KT:O_KT + 1024].rearrange("p (g q) -> p g q", g=2),
                    wt[:, O_VN:O_VN + 1024].bitcast(F16).rearrange(
                        "p (j c) -> p j c", j=4),
                    wt[:, O_LP:O_LP + 1024].rearrange("p (g q) -> p g q", g=2),
                )

            def emit_bg_exp_half(w, g, jc, h0, eT):
                """One head-pair of the QK jc-quarter (2 row-packed bf16
                MMs into one 2-bank PSUM tile) + its exp (Scalar/Vector)."""
                qT, kT, _, _ = views(wts[w])
                tag = "bigA" if h0 == 0 else "bigB"
                big = pp.tile([P, 1024], F32, name=f"b{tag[-1]}{w}{g}{jc}",
                              tag=tag, bufs=1)
                if w < NW - 1:
                    for hx in range(2):
                        hp = h0 + hx
                        nc.tensor.matmul(
                            big[:, 512 * hx:512 * (hx + 1)],
                            kT[32 * hp:32 * hp + 32, g, P * jc:P * (jc + 1)],
                            qT[32 * hp:32 * hp + 32, g, :],
                            start=True, stop=True,
                            tile_position=(32 * hp, 0))
                    ev = eT[:, 1024 * (h0 // 2):1024 * (h0 // 2) + 1024]
                    if _dve_exp_half(w, g, jc, h0):
                        nc.vector.tensor_scalar(
                            out=ev.bitcast(I16), in0=big[:],
                            scalar1=SCH_A, scalar2=SCH_B,
                            op0=mybir.AluOpType.mult,
                            op1=mybir.AluOpType.add)
                    else:
                        nc.scalar.activation(ev, big[:], Exp,
                                             bias=0.0, scale=1.0)
                else:
                    # shifted window: block-diagonal mask. keys of
                    # quarter jc only see queries qo..qo+256.
                    qo = 0 if jc < 2 else 256
                    for hx in range(2):
                        hp = h0 + hx
                        nc.tensor.matmul(
                            big[:, 512 * hx + qo:512 * hx + qo + 256],
                            kT[32 * hp:32 * hp + 32, g, P * jc:P * (jc + 1)],
                            qT[32 * hp:32 * hp + 32, g, qo:qo + 256],
                            start=True, stop=True,
                            tile_position=(32 * hp, 0))
                    bv = big[:].rearrange(
                        "p (h q) -> p h q", h=2)[:, :, qo:qo + 256]
                    ev = eT[:, 1024 * (h0 // 2):
                            1024 * (h0 // 2) + 1024].rearrange(
                        "p (h q) -> p h q", h=2)[:, :, qo:qo + 256]
                    nc.scalar.activation(ev, bv, Exp, bias=0.0, scale=1.0)

            pvsm_of = {}

            def emit_pvsm_chunk(w, g, jc, eTs, part="both"):
                """One jc-quarter of PV + denominator accumulation."""
                _, _, vn, _ = views(wts[w])
                if jc == 0:
                    pv = pp.tile([P, 512], F32, name=f"pv{w}{g}",
                                 tag="pv", bufs=1)
                    sm = pp.tile([P, 512], F32, name=f"sm{w}{g}",
                                 tag="sm", bufs=1)
                    pvsm_of[(w, g)] = (pv, sm)
                pv, sm = pvsm_of[(w, g)]
                if w < NW - 1:
                    qo, qn = 0, 512
                    st, sp_ = (jc == 0), (jc == 3)
                else:
                    qh, jx = jc // 2, jc % 2
                    qo, qn = 256 * qh, 256
                    st, sp_ = (jx == 0), (jx == 1)
                if part in ("pv", "both"):
                    for hp in range(4):
                        nc.tensor.matmul(
                            pv[32 * hp:32 * hp + 32, qo:qo + qn],
                            vn[:, jc, P * g + 32 * hp:P * g + 32 * hp + 32],
                            eTs[jc][:, 512 * hp + qo:512 * hp + qo + qn],
                            start=st, stop=sp_, tile_position=(0, 32 * hp))
                if part in ("sm", "both"):
                    for hp in range(4):
                        nc.tensor.matmul(
                            sm[32 * hp:32 * hp + 32, qo:qo + qn],
                            ones32[:],
                            eTs[jc][:, 512 * hp + qo:512 * hp + qo + qn],
                            start=st, stop=sp_, tile_position=(0, 32 * hp))

            mg_of = {}

            def emit_finish(w, g):
                """Normalize + merge LePE (per half, pipelined on DVE)."""
                _, _, _, lpT = views(wts[w])
                pv, sm = pvsm_of.pop((w, g))
                mg = sp.tile([P, 512], BF16, name=f"mg{w}{g}", tag="mg", bufs=4)
                for half in range(2):
                    sl = slice(256 * half, 256 * (half + 1))
                    rbs = sp.tile([P, 256], F32, name=f"rbs{w}{g}{half}",
                                  tag="rbs", bufs=3)
                    nc.vector.reciprocal_approx_fast(rbs[:], sm[:, sl])
                    mt = sp.tile([P, 256], BF16, name=f"mt{w}{g}{half}",
                                 tag="mt", bufs=2)
                    nc.vector.tensor_tensor(
                        out=mt[:], in0=pv[:, sl], in1=rbs[:],
                        op=mybir.AluOpType.mult)
                    nc.vector.tensor_tensor(
                        out=mg[:, sl], in0=mt[:], in1=lpT[:, g, sl],
                        op=mybir.AluOpType.add)
                mg_of[(w, g)] = mg

            ob_of = {}

            def emit_pj_piece(w, t4):
                """One token-quarter of the projection + bias/evacuate."""
                if t4 == 0:
                    ob_of[w] = sp.tile([P, 4, DIM], BF16, name=f"ob{w}",
                                       tag="ob", bufs=2)
                ob = ob_of[w]
                pj = pp.tile([P, DIM], F32, name=f"pj{w}{t4}",
                             tag="pj", bufs=2)
                nc.tensor.matmul(pj[:], mg_of[(w, 0)][:, P * t4:P * (t4 + 1)],
                                 pw_sb[:, 0, :], start=True, stop=False)
                nc.tensor.matmul(pj[:], mg_of[(w, 1)][:, P * t4:P * (t4 + 1)],
                                 pw_sb[:, 1, :], start=False, stop=True)
                nc.vector.tensor_tensor(
                    out=ob[:, t4, :], in0=pj[:], in1=pb_sb[:],
                    op=mybir.AluOpType.add)
                if t4 == 3:
                    nc.sync.dma_start(ov[w], ob_of.pop(w)[:])
                    del mg_of[(w, 0)], mg_of[(w, 1)]

            # fine-grained software pipeline: per jc-slot emit this pair's
            # QK+exp, then the PREVIOUS pair's PV/SM quarter, then (during
            # g=1 pairs) one proj piece of the previous window. No engine
            # queue head ever waits long, PE duty stays high.
            pairs = [(w, g) for w in range(NW) for g in range(2)]
            prev = None
            for w, g in pairs:
                if g == 0 and w + 1 < NW:   # prefetch next window's blob
                    nwt = sp.tile([P, WCOLS], BF16, name=f"wt{w + 1}",
                                  tag="wt", bufs=3)
                    nc.sync.dma_start(nwt[:], win[:][w + 1])
                    wts[w + 1] = nwt
                eTs = []
                for jc in range(4):
                    eT = sp.tile([P, 2048], F16, name=f"eT{w}{g}{jc}",
                                 tag="eT", bufs=12)
                    eTs.append(eT)
                    emit_bg_exp_half(w, g, jc, 0, eT)
                    if prev is not None:
                        emit_pvsm_chunk(prev[0], prev[1], jc, prev[2],
                                        part="pv")
                    emit_bg_exp_half(w, g, jc, 2, eT)
                    if prev is not None:
                        emit_pvsm_chunk(prev[0], prev[1], jc, prev[2],
                                        part="sm")
                    if g == 1 and w >= 1:
                        emit_pj_piece(w - 1, jc)
                if prev is not None:
                    emit_finish(prev[0], prev[1])
                prev = (w, g, eTs)
            for jc in range(4):
                emit_pvsm_chunk(prev[0], prev[1], jc, prev[2])
            emit_finish(prev[0], prev[1])
            for t4 in range(4):
                emit_pj_piece(NW - 1, t4)
    return nc


_CACHE = {}


def _get_nc():
    if "nc" not in _CACHE:
        nc = build_nc()
        nc.finalize()
        _CACHE["nc"] = nc
    return _CACHE["nc"]


def _host_lepe(v_win, conv_w, conv_b):
    """Depthwise 3x3 conv on [B, NW, C, 64, 8] window images (host, fp32).

    Each 64x8 window is zero-padded independently, matching the
    reference's per-window lax.conv on [B*nW, C, Hsp, Wsp]."""
    Bx, nw, C, H, W = v_win.shape
    pad = np.zeros((Bx, nw, C, H + 2, W + 2), np.float32)
    pad[:, :, :, 1:-1, 1:-1] = v_win
    out = np.broadcast_to(
        conv_b[None, None, :, None, None], v_win.shape).copy()
    cw = conv_w.reshape(C, 3, 3)
    for dy in range(3):
        for dx in range(3):
            out += cw[None, None, :, dy, dx, None, None] * \
                pad[:, :, :, dy:dy + H, dx:dx + W]
    return out


def _host_prep(qkv, scale, proj_w, proj_b, conv_w, conv_b):
    """Per-core input maps: all device layouts built host-side."""
    scale_v = float(np.asarray(scale).reshape(-1)[0])
    q_all = np.asarray(qkv[0], np.float32) * scale_v
    k_all = np.asarray(qkv[1], np.float32)
    v_all = np.asarray(qkv[2], np.float32)
    conv_w_h = np.asarray(conv_w, np.float32)
    conv_b_h = np.asarray(conv_b, np.float32)

    # weights (shared across cores). conv bias is folded into the lepe
    # plane itself (host conv adds it), so proj bias is just proj_b.
    pw_h = np.ascontiguousarray(np.asarray(proj_w).T.reshape(2, P, DIM)
                                .transpose(1, 0, 2).reshape(P, 2 * DIM)
                                ).astype(ml_dtypes.bfloat16)
    pb_h = np.ascontiguousarray(np.broadcast_to(
        np.asarray(proj_b, np.float32)[None, :], (P, DIM)))

    # token reorder: l = h*64 + w*8 + s  ->  window w, t' = s*64 + h
    def to_win(x):
        xw = x.reshape(B, RESO, NW, STRIPE, DIM)          # [b, h, w, s, c]
        return np.ascontiguousarray(xw.transpose(0, 2, 3, 1, 4)).reshape(
            B, NW, WIN, DIM)                               # [b, w, s*64+h, c]

    qw = to_win(q_all)
    kw = to_win(k_all)
    vw = to_win(v_all)

    # lepe: per-window depthwise conv; vw is [b, w, (s h), c]
    v_win = vw.reshape(B, NW, STRIPE, RESO, DIM).transpose(0, 1, 4, 3, 2)
    lepe = _host_lepe(v_win, conv_w_h, conv_b_h)      # [b, w, c, h, s]
    lw = np.ascontiguousarray(lepe.transpose(0, 1, 4, 3, 2)).reshape(
        B, NW, WIN, DIM)                               # [b, w, (s h), c]

    # fused per-window blob [B, NW, P, WCOLS]: bf16 planes for qT/kT/lepeT,
    # fp16 bits for the vn plane (PV runs in fp16 to match the Schraudolph
    # fp16 eT tiles).
    blob = np.zeros((B, NW, P, WCOLS), np.uint16)

    def bf16_bits(x):
        return x.astype(ml_dtypes.bfloat16).view(np.uint16)

    # qT/kT/lepeT: [p = ch within g, g*512 + t']
    for off, src in ((O_QT, qw), (O_KT, kw), (O_LP, lw)):
        t = src.transpose(0, 1, 3, 2).reshape(B, NW, 2, P, WIN)
        blob[:, :, :, off:off + 1024] = bf16_bits(
            t.transpose(0, 1, 3, 2, 4).reshape(B, NW, P, 1024))
    # vn: [p = t' % 128, (jc, ch)] as fp16
    blob[:, :, :, O_VN:O_VN + 1024] = vw.reshape(
        B, NW, 4, P, DIM).transpose(0, 1, 3, 2, 4).reshape(
        B, NW, P, 1024).astype(np.float16).view(np.uint16)
    blob_bf = blob.view(ml_dtypes.bfloat16)

    in_maps = []
    for b in range(B):
        in_maps.append({
            "win": np.ascontiguousarray(blob_bf[b]),
            "pw": pw_h, "pb": pb_h,
        })
    return in_maps


LAST_RESULTS = None


def kernel(qkv, scale, proj_w, proj_b, conv_w, conv_b):
    global LAST_RESULTS
    from concourse.bass_utils import run_bass_kernel_spmd
    nc = _get_nc()
    in_maps = _host_prep(qkv, scale, proj_w, proj_b, conv_w, conv_b)
    res = run_bass_kernel_spmd(nc, in_maps, core_ids=list(range(B)))
    LAST_RESULTS = res
    outs = []
    for b in range(B):
        o = np.asarray(res.results[b]["out"]).astype(np.float32)
        # device layout: [w, p, t4, c] with t' = t4*128 + p; t' = s*64 + h
        o = o.reshape(NW, P, 4, DIM).transpose(0, 2, 1, 3)   # [w, t4, p, c]
        o = o.reshape(NW, STRIPE, RESO, DIM)                 # [w, s, h, c]
        o = o.transpose(2, 0, 1, 3).reshape(L, DIM)          # [h*64+w*8+s, c]
        outs.append(o)
    return np.stack(outs, axis=0)


# revision 19
# speedup vs baseline: 1.3721x; 1.3721x over previous
"""CSwin vertical-stripe window attention (sparse_attention) on 8 TRN2 cores.

Sharding: data-parallel over batch B=8 (one image per NeuronCore). No
collectives. Per-core kernel computes windowed attention + output
projection for one [4096, 256] image; the tiny LePE depthwise 3x3 conv
(0.7% of FLOPs) is folded host-side into a per-window additive plane.

v6 design (exp split across Scalar AND Vector engines):
 - All input layouts prepared host-side: qT/kT/lepeT bf16, vn fp16 --
   one fused [128, 4096] DMA per window.
 - QK^T bf16, 4-head row-packed, split across TWO 2-bank PSUM tiles
   (heads 01 -> bigA, heads 23 -> bigB); fine-grained software pipeline
   emits, per jc-slot, this pair's QK+exp then the previous pair's
   PV/SM quarter and a proj piece, so no engine queue head ever blocks.
 - exp: Scalar-engine ACTIVATE for most tiles; for jc==1 tiles a
   single-instruction DVE Schraudolph (fp16-bit trick: round(x*a+b) as
   int16 IS the fp16 exp, ~3% max rel err, mean bias cancels in
   softmax) offloads ~20% of the exp work to the Vector engine.
 - PV + denominators col-packed fp16; reciprocal_approx_fast on DVE.
 - Window 7 (shifted stripes) is block-diagonal: masked quadrants are
   never computed (N=256 matmuls, strided Exp).
 - proj bias added by the DVE PSUM-evacuation op; output bf16
   window-major contiguous, host un-permutes.
"""
import numpy as np
import ml_dtypes

import concourse.bass as bass
import concourse.bacc as bacc
import concourse.mybir as mybir
import concourse.tile as tile

RESO, STRIPE, DIM, NH, HD = 64, 8, 256, 8, 32
B, L, WIN, NW = 8, RESO * RESO, RESO * STRIPE, RESO // STRIPE
P = 128
F32, BF16 = mybir.dt.float32, mybir.dt.bfloat16
F16, I16 = mybir.dt.float16, mybir.dt.int16

# fused per-window input blob offsets (16-bit elements, per partition)
O_QT, O_KT, O_VN, O_LP = 0, 1024, 2048, 3072
WCOLS = 4096

Exp = mybir.ActivationFunctionType.Exp
# Schraudolph fp16 exp: fp16_bits(e^x) ~ round(x * SCH_A + SCH_B)
SCH_A, SCH_B = 1477.3197218702985, 15315.5


def _dve_exp_half(w, g, jc, h0):
    """Which exp halves run on the Vector engine instead of Scalar.

    B-halves only: the offload's pipeline win is freeing bigB early so
    the next QK group unblocks; A-half offloads only added Vector-engine
    contention (Scalar has slack under the PE ceiling)."""
    return w != NW - 1 and jc == 1 and h0 == 2


def build_nc():
    nc = bacc.Bacc("TRN2", target_bir_lowering=False, debug=False)
    win = nc.declare_dram_parameter("win", [NW, P, WCOLS], BF16, isOutput=False)
    pw = nc.declare_dram_parameter("pw", [P, 2 * DIM], BF16, isOutput=False)
    pb = nc.declare_dram_parameter("pb", [P, DIM], F32, isOutput=False)
    out = nc.declare_dram_parameter("out", [L, DIM], BF16, isOutput=True)

    # output view: [w, p, (t4, c)] with window token t' = t4*128 + p
    ov = out[:].rearrange("(w p t) c -> w p (t c)", w=NW, p=P, t=4)

    with tile.TileContext(nc) as tc:
        with tc.tile_pool(name="const", bufs=1) as cp, \
             tc.tile_pool(name="sb", bufs=1) as sp, \
             tc.tile_pool(name="ps", bufs=1, space="PSUM") as pp:
            # ---- first window's qk plane goes out before anything else ----
            wts = {}
            wt0 = sp.tile([P, WCOLS], BF16, name="wt0", tag="wt", bufs=3)
            nc.sync.dma_start(wt0[:, :2048], win[:][0][:, :2048])
            nc.sync.dma_start(wt0[:, 2048:], win[:][0][:, 2048:])
            wts[0] = wt0

            # ---- constants ----
            ones32 = cp.tile([P, 32], F16, name="ones32")
            nc.vector.memset(ones32[:], 1.0)
            pw_sb = cp.tile([P, 2, DIM], BF16, name="pw_sb")
            nc.sync.dma_start(pw_sb[:], pw[:].rearrange("p (g c) -> p g c", g=2))
            pb_sb = cp.tile([P, DIM], F32, name="pb_sb")
            nc.sync.dma_start(pb_sb[:], pb[:])

            def views(wt):
                return (
                    wt[:, O_QT:O_QT + 1024].rearrange("p (g q) -> p g q", g=2),
                    wt[:, O_KT:O_KT + 1024].rearrange("p (g q) -> p g q", g=2),
                    wt[:, O_VN:O_VN + 1024].bitcast(F16).rearrange(
                        "p (j c) -> p j c", j=4),
                    wt[:, O_LP:O_LP + 1024].rearrange("p (g q) -> p g q", g=2),
                )

            def emit_bg_exp(w, g, jc):
                """One QK jc-quarter (4 row-packed bf16 MMs into the A/B
                PSUM pair) followed by its two exps (Scalar or Vector)."""
                qT, kT, _, _ = views(wts[w])
                bigA = pp.tile([P, 1024], F32, name=f"bA{w}{g}{jc}",
                               tag="bigA", bufs=1)
                bigB = pp.tile([P, 1024], F32, name=f"bB{w}{g}{jc}",
                               tag="bigB", bufs=1)
                eT = sp.tile([P, 2048], F16, name=f"eT{w}{g}{jc}",
                             tag="eT", bufs=12)
                halves = ((bigA, 0), (bigB, 2))
                if w < NW - 1:
                    for big, h0 in halves:
                        for hx in range(2):
                            hp = h0 + hx
                            nc.tensor.matmul(
                                big[:, 512 * hx:512 * (hx + 1)],
                                kT[32 * hp:32 * hp + 32, g,
                                   P * jc:P * (jc + 1)],
                                qT[32 * hp:32 * hp + 32, g, :],
                                start=True, stop=True,
                                tile_position=(32 * hp, 0))
                    for big, h0 in halves:
                        ev = eT[:, 1024 * (h0 // 2):1024 * (h0 // 2) + 1024]
                        if _dve_exp_half(w, g, jc, h0):
                            nc.vector.tensor_scalar(
                                out=ev.bitcast(I16), in0=big[:],
                                scalar1=SCH_A, scalar2=SCH_B,
                                op0=mybir.AluOpType.mult,
                                op1=mybir.AluOpType.add)
                        else:
                            nc.scalar.activation(ev, big[:], Exp,
                                                 bias=0.0, scale=1.0)
                else:
                    # shifted window: block-diagonal mask. keys of
                    # quarter jc only see queries qo..qo+256.
                    qo = 0 if jc < 2 else 256
                    for big, h0 in halves:
                        for hx in range(2):
                            hp = h0 + hx
                            nc.tensor.matmul(
                                big[:, 512 * hx + qo:512 * hx + qo + 256],
                                kT[32 * hp:32 * hp + 32, g,
                                   P * jc:P * (jc + 1)],
                                qT[32 * hp:32 * hp + 32, g, qo:qo + 256],
                                start=True, stop=True,
                                tile_position=(32 * hp, 0))
                    for big, h0 in halves:
                        bv = big[:].rearrange(
                            "p (h q) -> p h q", h=2)[:, :, qo:qo + 256]
                        ev = eT[:, 1024 * (h0 // 2):
                                1024 * (h0 // 2) + 1024].rearrange(
                            "p (h q) -> p h q", h=2)[:, :, qo:qo + 256]
                        nc.scalar.activation(ev, bv, Exp,
                                             bias=0.0, scale=1.0)
                return eT

            pvsm_of = {}

            def emit_pvsm_chunk(w, g, jc, eTs):
                """One jc-quarter of PV + denominator accumulation."""
                _, _, vn, _ = views(wts[w])
                if jc == 0:
                    pv = pp.tile([P, 512], F32, name=f"pv{w}{g}",
                                 tag="pv", bufs=1)
                    sm = pp.tile([P, 512], F32, name=f"sm{w}{g}",
                                 tag="sm", bufs=1)
                    pvsm_of[(w, g)] = (pv, sm)
                pv, sm = pvsm_of[(w, g)]
                if w < NW - 1:
                    qo, qn = 0, 512
                    st, sp_ = (jc == 0), (jc == 3)
                else:
                    qh, jx = jc // 2, jc % 2
                    qo, qn = 256 * qh, 256
                    st, sp_ = (jx == 0), (jx == 1)
                for hp in range(4):
                    nc.tensor.matmul(
                        pv[32 * hp:32 * hp + 32, qo:qo + qn],
                        vn[:, jc, P * g + 32 * hp:P * g + 32 * hp + 32],
                        eTs[jc][:, 512 * hp + qo:512 * hp + qo + qn],
                        start=st, stop=sp_, tile_position=(0, 32 * hp))
                for hp in range(4):
                    nc.tensor.matmul(
                        sm[32 * hp:32 * hp + 32, qo:qo + qn],
                        ones32[:],
                        eTs[jc][:, 512 * hp + qo:512 * hp + qo + qn],
                        start=st, stop=sp_, tile_position=(0, 32 * hp))

            mg_of = {}

            def emit_finish(w, g):
                """Normalize + merge LePE (per half, pipelined on DVE)."""
                _, _, _, lpT = views(wts[w])
                pv, sm = pvsm_of.pop((w, g))
                mg = sp.tile([P, 512], BF16, name=f"mg{w}{g}", tag="mg", bufs=4)
                for half in range(2):
                    sl = slice(256 * half, 256 * (half + 1))
                    rbs = sp.tile([P, 256], F32, name=f"rbs{w}{g}{half}",
                                  tag="rbs", bufs=3)
                    nc.vector.reciprocal_approx_fast(rbs[:], sm[:, sl])
                    mt = sp.tile([P, 256], BF16, name=f"mt{w}{g}{half}",
                                 tag="mt", bufs=2)
                    nc.vector.tensor_tensor(
                        out=mt[:], in0=pv[:, sl], in1=rbs[:],
                        op=mybir.AluOpType.mult)
                    nc.vector.tensor_tensor(
                        out=mg[:, sl], in0=mt[:], in1=lpT[:, g, sl],
                        op=mybir.AluOpType.add)
                mg_of[(w, g)] = mg

            ob_of = {}

            def emit_pj_piece(w, t4):
                """One token-quarter of the projection + bias/evacuate."""
                if t4 == 0:
                    ob_of[w] = sp.tile([P, 4, DIM], BF16, name=f"ob{w}",
                                       tag="ob", bufs=2)
                ob = ob_of[w]
                pj = pp.tile([P, DIM], F32, name=f"pj{w}{t4}",
                             tag="pj", bufs=2)
                nc.tensor.matmul(pj[:], mg_of[(w, 0)][:, P * t4:P * (t4 + 1)],
                                 pw_sb[:, 0, :], start=True, stop=False)
                nc.tensor.matmul(pj[:], mg_of[(w, 1)][:, P * t4:P * (t4 + 1)],
                                 pw_sb[:, 1, :], start=False, stop=True)
                nc.vector.tensor_tensor(
                    out=ob[:, t4, :], in0=pj[:], in1=pb_sb[:],
                    op=mybir.AluOpType.add)
                if t4 == 3:
                    nc.sync.dma_start(ov[w], ob_of.pop(w)[:])
                    del mg_of[(w, 0)], mg_of[(w, 1)]

            # fine-grained software pipeline: per jc-slot emit this pair's
            # QK+exp, then the PREVIOUS pair's PV/SM quarter, then (during
            # g=1 pairs) one proj piece of the previous window. No engine
            # queue head ever waits long, PE duty stays high.
            pairs = [(w, g) for w in range(NW) for g in range(2)]
            prev = None
            for w, g in pairs:
                if g == 0 and w + 1 < NW:   # prefetch next window's blob
                    nwt = sp.tile([P, WCOLS], BF16, name=f"wt{w + 1}",
                                  tag="wt", bufs=3)
                    nc.sync.dma_start(nwt[:], win[:][w + 1])
                    wts[w + 1] = nwt
                eTs = []
                for jc in range(4):
                    eTs.append(emit_bg_exp(w, g, jc))
                    if prev is not None:
                        emit_pvsm_chunk(prev[0], prev[1], jc, prev[2])
                    if g == 1 and w >= 1:
                        emit_pj_piece(w - 1, jc)
                if prev is not None:
                    emit_finish(prev[0], prev[1])
                prev = (w, g, eTs)
            for jc in range(4):
                emit_pvsm_chunk(prev[0], prev[1], jc, prev[2])
            emit_finish(prev[0], prev[1])
            for t4 in range(4):
                emit_pj_piece(NW - 1, t4)
    return nc


_CACHE = {}


def _get_nc():
    if "nc" not in _CACHE:
        nc = build_nc()
        nc.finalize()
        _CACHE["nc"] = nc
    return _CACHE["nc"]


def _host_lepe(v_win, conv_w, conv_b):
    """Depthwise 3x3 conv on [B, NW, C, 64, 8] window images (host, fp32).

    Each 64x8 window is zero-padded independently, matching the
    reference's per-window lax.conv on [B*nW, C, Hsp, Wsp]."""
    Bx, nw, C, H, W = v_win.shape
    pad = np.zeros((Bx, nw, C, H + 2, W + 2), np.float32)
    pad[:, :, :, 1:-1, 1:-1] = v_win
    out = np.broadcast_to(
        conv_b[None, None, :, None, None], v_win.shape).copy()
    cw = conv_w.reshape(C, 3, 3)
    for dy in range(3):
        for dx in range(3):
            out += cw[None, None, :, dy, dx, None, None] * \
                pad[:, :, :, dy:dy + H, dx:dx + W]
    return out


def _host_prep(qkv, scale, proj_w, proj_b, conv_w, conv_b):
    """Per-core input maps: all device layouts built host-side."""
    scale_v = float(np.asarray(scale).reshape(-1)[0])
    q_all = np.asarray(qkv[0], np.float32) * scale_v
    k_all = np.asarray(qkv[1], np.float32)
    v_all = np.asarray(qkv[2], np.float32)
    conv_w_h = np.asarray(conv_w, np.float32)
    conv_b_h = np.asarray(conv_b, np.float32)

    # weights (shared across cores). conv bias is folded into the lepe
    # plane itself (host conv adds it), so proj bias is just proj_b.
    pw_h = np.ascontiguousarray(np.asarray(proj_w).T.reshape(2, P, DIM)
                                .transpose(1, 0, 2).reshape(P, 2 * DIM)
                                ).astype(ml_dtypes.bfloat16)
    pb_h = np.ascontiguousarray(np.broadcast_to(
        np.asarray(proj_b, np.float32)[None, :], (P, DIM)))

    # token reorder: l = h*64 + w*8 + s  ->  window w, t' = s*64 + h
    def to_win(x):
        xw = x.reshape(B, RESO, NW, STRIPE, DIM)          # [b, h, w, s, c]
        return np.ascontiguousarray(xw.transpose(0, 2, 3, 1, 4)).reshape(
            B, NW, WIN, DIM)                               # [b, w, s*64+h, c]

    qw = to_win(q_all)
    kw = to_win(k_all)
    vw = to_win(v_all)

    # lepe: per-window depthwise conv; vw is [b, w, (s h), c]
    v_win = vw.reshape(B, NW, STRIPE, RESO, DIM).transpose(0, 1, 4, 3, 2)
    lepe = _host_lepe(v_win, conv_w_h, conv_b_h)      # [b, w, c, h, s]
    lw = np.ascontiguousarray(lepe.transpose(0, 1, 4, 3, 2)).reshape(
        B, NW, WIN, DIM)                               # [b, w, (s h), c]

    # fused per-window blob [B, NW, P, WCOLS]: bf16 planes for qT/kT/lepeT,
    # fp16 bits for the vn plane (PV runs in fp16 to match the Schraudolph
    # fp16 eT tiles).
    blob = np.zeros((B, NW, P, WCOLS), np.uint16)

    def bf16_bits(x):
        return x.astype(ml_dtypes.bfloat16).view(np.uint16)

    # qT/kT/lepeT: [p = ch within g, g*512 + t']
    for off, src in ((O_QT, qw), (O_KT, kw), (O_LP, lw)):
        t = src.transpose(0, 1, 3, 2).reshape(B, NW, 2, P, WIN)
        blob[:, :, :, off:off + 1024] = bf16_bits(
            t.transpose(0, 1, 3, 2, 4).reshape(B, NW, P, 1024))
    # vn: [p = t' % 128, (jc, ch)] as fp16
    blob[:, :, :, O_VN:O_VN + 1024] = vw.reshape(
        B, NW, 4, P, DIM).transpose(0, 1, 3, 2, 4).reshape(
        B, NW, P, 1024).astype(np.float16).view(np.uint16)
    blob_bf = blob.view(ml_dtypes.bfloat16)

    in_maps = []
    for b in range(B):
        in_maps.append({
            "win": np.ascontiguousarray(blob_bf[b]),
            "pw": pw_h, "pb": pb_h,
        })
    return in_maps


LAST_RESULTS = None


def kernel(qkv, scale, proj_w, proj_b, conv_w, conv_b):
    global LAST_RESULTS
    from concourse.bass_utils import run_bass_kernel_spmd
    nc = _get_nc()
    in_maps = _host_prep(qkv, scale, proj_w, proj_b, conv_w, conv_b)
    res = run_bass_kernel_spmd(nc, in_maps, core_ids=list(range(B)))
    LAST_RESULTS = res
    outs = []
    for b in range(B):
        o = np.asarray(res.results[b]["out"]).astype(np.float32)
        # device layout: [w, p, t4, c] with t' = t4*128 + p; t' = s*64 + h
        o = o.reshape(NW, P, 4, DIM).transpose(0, 2, 1, 3)   # [w, t4, p, c]
        o = o.reshape(NW, STRIPE, RESO, DIM)                 # [w, s, h, c]
        o = o.transpose(2, 0, 1, 3).reshape(L, DIM)          # [h*64+w*8+s, c]
        outs.append(o)
    return np.stack(outs, axis=0)
